# revision 47
# baseline (speedup 1.0000x reference)
"""Trainium2 Bass kernel for CheemsNonWoAttn (GQA attention block, no Wo).

Sharding: 8 cores = batch(2) x kv-head-pair(4). Each core handles one batch
element and 2 of the 8 kv heads (GQA: Q and K are repeated identically across
the 4 groups, so only 8 unique softmax matrices exist; V uses all 32 heads).

Per-core device program:
  Qt/Kt = Wq^T X^T, Wk^T X^T    (d on partitions; N=512 matmuls, K accum=16)
  RoPE on DVE (partition-shifted multiplies, sign folded into sin table)
  V = X Wv                      (natural [s, 512] layout)
  scores^T[k, q] = Kt^T Qt      (per head, K=64, row-tiled across array halves)
  E = exp(scores) on ACT        (no max subtraction: |scores| <~ 6, safe)
  causal mask: multiply 128x128 diagonal blocks by triangular mask (DVE)
  out = E^T V with a ones-column in V producing the softmax denominator
  out /= denom (DVE reciprocal + tensor_scalar)
"""
import os
import sys
import types
from contextlib import ExitStack

for _p in ("/opt/trn_rl_repo", "/root/.axon_site/_ro/trn_rl_repo"):
    if os.path.isdir(_p) and _p not in sys.path:
        sys.path.append(_p)

import numpy as np
import ml_dtypes

import concourse.bass as bass
import concourse.tile as tile
from concourse import mybir
from concourse.bass_utils import run_bass_kernel_spmd
from concourse.vector_clock import ScopedClock

# ---------------------------------------------------------------------------
# Patch 1: walrus rejects Drain instructions with >1 sync wait (CTRL ops have
# a single wait slot). Split the TileContext exit drain's waits across extra
# SP nops, one wait each.
def _patched_drain_and_barrier(self, tick_clock, wait_clock):
    nc = self.nc
    drain_bi = nc.sync.drain()
    wait_clock.add_sem_waits(drain_bi.ins, ScopedClock({None: tick_clock.global_clock}))
    inst = drain_bi.ins
    si = inst.sync_info
    if si is not None and si.on_wait is not None and len(si.on_wait) > 1:
        waits = list(si.on_wait)
        inst.sync_info = mybir.SyncInfo(
            on_wait=waits[:1],
            on_update=list(si.on_update) if si.on_update else [],
        )
        for w in waits[1:]:
            nbi = nc.sync.nop()
            nbi.ins.sync_info = mybir.SyncInfo(on_wait=[w], on_update=[])
    nc.all_engine_barrier()
    assert self.sems is not None
    popped = nc._tile_sem_poison_stack.pop()
    assert popped is self._sem_poison
    nc.clear_and_free_semaphores(list(self.sems.allocated().values()))
    nc.all_engine_barrier()


tile.TileContext._drain_and_barrier = _patched_drain_and_barrier


def _legalize_waits(nc):
    """This walrus build accepts at most one sync-wait per instruction.
    Split any instruction carrying N>1 waits into N-1 preceding same-engine
    nops (engines are in-order, so semantics are preserved)."""
    uid = 0
    for f in nc.m.functions:
        for blk in f.blocks:
            insts = list(blk.instructions)
            out, changed = [], False
            for inst in insts:
                si = getattr(inst, "sync_info", None)
                if si is not None and si.on_wait is not None and len(si.on_wait) > 1:
                    waits = list(si.on_wait)
                    for w in waits[:-1]:
                        uid += 1
                        out.append(mybir.InstNoOp(
                            name=f"{inst.name}_lw{uid}",
                            engine=inst.engine,
                            sync_info=mybir.SyncInfo(on_wait=[w], on_update=[]),
                            bass_nofuse=True,
                        ))
                    inst.sync_info = mybir.SyncInfo(
                        on_wait=waits[-1:],
                        on_update=list(si.on_update) if si.on_update else [],
                    )
                    changed = True
                out.append(inst)
            if changed:
                blk.instructions = out


def _dedup_ldweights(nc):
    """Clear the self-load on matmuls whose stationary operand AP is identical
    to the immediately preceding matmul in the final PE stream (walrus runs
    with ldw-opt disabled, so it reloads weights for every matmul otherwise)."""
    if os.environ.get("CHEEMS_NO_LDW_DEDUP"):
        return
    for f in nc.m.functions:
        for blk in f.blocks:
            prev_key = None
            for inst in blk.instructions:
                if not isinstance(inst, mybir.InstMatmult):
                    if isinstance(inst, mybir.InstLdweights):
                        prev_key = None
                    continue
                key = (repr(inst.ins[1]), inst.perf_mode, inst.is_transpose,
                       repr(inst.tile_position))
                if prev_key is not None and key == prev_key:
                    inst.ldweights = False
                prev_key = key


# Patch 2 (optional, for tracing): recreate the antenv.axon_hooks shim so
# run_bass_kernel_spmd(trace=True) can capture NTFF profiles under axon.
def _install_ntff_hook():
    try:
        if "antenv.axon_hooks" in sys.modules:
            return
        import antenv
        from trn_agent_boot.trn_boot import _ntff_profile_via_ctypes

        hook = _ntff_profile_via_ctypes("/opt/axon/libaxon_pjrt.so")
        mod = types.ModuleType("antenv.axon_hooks")
        mod._hook = hook
        mod.get_axon_ntff_profile_hook = lambda: mod._hook

        def _set(h):
            mod._hook = h

        mod.set_axon_ntff_profile_hook = _set
        sys.modules["antenv.axon_hooks"] = mod
        antenv.axon_hooks = mod
    except Exception:
        pass


# ---------------------------------------------------------------------------
B, S, HID = 2, 2048, 2048
NH, G = 32, 4
HD = 64          # head dim
HKV = 8          # kv heads
THETA = 10000.0
P = 128          # partitions
NKT = HID // P   # 16 k-tiles over the contraction dim
NST = S // P     # 16 s-tiles
NCH = 4          # s-chunks of 512
CH = 512
DV = 512         # v columns per core (8 heads x 64)
VROW = 528       # v tile row: [256 v | 1 one | 7 pad] x 2 heads
VOFF = 264

F32 = mybir.dt.float32
BF16 = mybir.dt.bfloat16

CW = S + 2 * P  # combined per-k-tile row: [xt 2048 | wq 128 | wk 128]

_CACHE = {}
LAST_RESULTS = None


def _mask_eng(nc):
    return nc.vector if os.environ.get("CHEEMS_MASK_VEC") else nc.gpsimd


def _build():
    nc = bass.Bass("TRN2")
    # Inputs host-prearranged to [P, ...] contiguous layouts: one 4KB+ row per
    # partition -> few large DMA descriptors. xt/wq/wk are fused into one
    # per-k-tile block ("comb": [xt 2048 | wq 128 | wk 128] per partition row)
    # so one fat DMA delivers everything the QK-proj k-iteration needs --
    # the PE can start at the first tile's arrival and never waits on a
    # separately-queued weight tensor.
    d_comb = nc.declare_dram_parameter("comb", [P, NKT * CW], BF16, isOutput=False)
    d_wv = nc.declare_dram_parameter("wv", [P, NKT * DV], BF16, isOutput=False)
    d_cos = nc.declare_dram_parameter("cost", [P, S], BF16, isOutput=False)
    d_sin = nc.declare_dram_parameter("sint", [P, S], BF16, isOutput=False)
    d_tri = nc.declare_dram_parameter("tri", [P, P], BF16, isOutput=False)
    d_perm = nc.declare_dram_parameter("perm", [P, P], BF16, isOutput=False)
    d_out = nc.declare_dram_parameter("out", [S, DV], BF16, isOutput=True)

    with tile.TileContext(nc) as tc, ExitStack() as ctx:
        pers = ctx.enter_context(tc.tile_pool(name="pers", bufs=1))
        epool = ctx.enter_context(tc.tile_pool(name="epool", bufs=56))
        work = ctx.enter_context(tc.tile_pool(name="work", bufs=1))
        outp = ctx.enter_context(tc.tile_pool(name="outp", bufs=3))
        psum = ctx.enter_context(tc.tile_pool(name="psum", bufs=8, space="PSUM"))

        def ps_tile(name):
            return psum.tile([P, CH], F32, tag="ps", bufs=8, name=name)

        # --- persistent tiles
        comb = pers.tile([P, NKT, CW], BF16, tag="comb")
        wv_sb = pers.tile([P, NKT, DV], BF16, tag="wv")
        cos_sb = pers.tile([P, S], BF16, tag="cos")
        sin_sb = pers.tile([P, S], BF16, tag="sin")
        tri_sb = pers.tile([P, P], BF16, tag="tri")

        # DMA order tuned against consumption: the QK-proj k-loop eats one
        # comb tile per ~1.7us (warm) while DMA delivers one per ~1.5us, so
        # the 16 comb tiles stream first (any gap stalls the PE *and* risks a
        # HAM idle-window re-throttle). wv/cos/sin/tri are needed only at the
        # phase-2 boundary (~38us) and ride in the tail slack, interleaved so
        # nothing arrives after its first use.
        perm_sb = pers.tile([P, P], BF16, tag="perm")
        comb_r = d_comb[:].rearrange("p (kt w) -> p kt w", w=CW)
        wv_r = d_wv[:].rearrange("p (kt n) -> p kt n", n=DV)
        nc.sync.dma_start(out=perm_sb[:], in_=d_perm[:])
        for k in range(14):
            nc.sync.dma_start(out=comb[:, k, :], in_=comb_r[:, k, :])
        nc.sync.dma_start(out=wv_sb[:, 0:4, :], in_=wv_r[:, 0:4, :])
        nc.sync.dma_start(out=comb[:, 14, :], in_=comb_r[:, 14, :])
        nc.sync.dma_start(out=wv_sb[:, 4:8, :], in_=wv_r[:, 4:8, :])
        nc.sync.dma_start(out=comb[:, 15, :], in_=comb_r[:, 15, :])
        nc.sync.dma_start(out=wv_sb[:, 8:12, :], in_=wv_r[:, 8:12, :])
        nc.sync.dma_start(out=cos_sb[:], in_=d_cos[:])
        nc.sync.dma_start(out=wv_sb[:, 12:16, :], in_=wv_r[:, 12:16, :])
        nc.sync.dma_start(out=sin_sb[:], in_=d_sin[:])
        nc.sync.dma_start(out=tri_sb[:], in_=d_tri[:])

        qt = pers.tile([P, S], BF16, tag="qt")
        kt = pers.tile([P, S], BF16, tag="kt")
        qraw = pers.tile([P, S], BF16, tag="qraw")
        kraw = pers.tile([P, S], BF16, tag="kraw")
        v_sb = [pers.tile([P, VROW], BF16, tag=f"v{t}", name=f"v{t}") for t in range(NST)]
        junk = pers.tile([P, 256], BF16, tag="junk")
        # gpsimd's preamble ends ~7.2us (before any DMA lands): its memset
        # gives the warmup matmuls an operand ~2us earlier than the first DMA.
        nc.gpsimd.memset(junk[:], 0.5)

        # ones columns for the softmax-denominator trick: written once, the
        # per-tile V copies never touch them.
        for t in range(NST):
            nc.vector.memset(v_sb[t][:, 256:257], 1.0)
            nc.vector.memset(v_sb[t][:, VOFF + 256:VOFF + 257], 1.0)

        # --- warmup: dummy matmuls wake the PE HAM clock gate (idle default is
        # 1.2GHz; ~3.4us of activity unlocks 2.4GHz) while the first DMAs land.
        # PE preamble ends ~7.3us and the first comb tile lands ~12.9us; the
        # warmup must keep the PE continuously busy across that span (any idle
        # window resets the HAM clock gate) so phase 1 starts at 2.4GHz.
        n_warm = int(os.environ.get("CHEEMS_WARM", "24"))
        if n_warm:
            ps_warm = ps_tile("ps_warm")
            for _ in range(n_warm):
                nc.tensor.matmul(ps_warm[:, 0:256], lhsT=junk[:, 0:P], rhs=junk[:],
                                 start=True, stop=True, skip_group_check=True)

        # --- phase 1: Q and K projections interleaved per xt tile (8 MMs per
        # DMA arrival keeps PE duty high while the xt stream lands).
        pq = [ps_tile(f"pq{c}") for c in range(NCH)]
        pk = [ps_tile(f"pk{c}") for c in range(NCH)]
        for k in range(NKT):
            for c in range(NCH):
                nc.tensor.matmul(pq[c][:], lhsT=comb[:, k, bass.ds(S, P)],
                                 rhs=comb[:, k, bass.ts(c, CH)],
                                 start=(k == 0), stop=(k == NKT - 1), skip_group_check=True)
            for c in range(NCH):
                nc.tensor.matmul(pk[c][:], lhsT=comb[:, k, bass.ds(S + P, P)],
                                 rhs=comb[:, k, bass.ts(c, CH)],
                                 start=(k == 0), stop=(k == NKT - 1), skip_group_check=True)

        # --- drain the 8 psum banks fast (scalar: q, vector: k in parallel)
        # so V-proj matmuls restart immediately. The rotate_half partition
        # shift is a signed-swap permutation: done on the PE (lhsT = swap
        # matrix, ~0.2us per chunk, emitted interleaved with the first V-proj
        # tiles) with the sin-multiply reading the swapped chunk straight from
        # PSUM. An SBUF->SBUF DMA shift would queue behind the whole 12.5MB
        # input stream and delay RoPE (and the scores behind it) by ~7us.
        for c in range(NCH):
            cs = bass.ts(c, CH)
            nc.scalar.copy(qraw[:, cs], pq[c][:])
            nc.vector.tensor_copy(kraw[:, cs], pk[c][:])

        def emit_rope_chunk(c):
            cs = bass.ts(c, CH)
            for src, dst in ((kraw, kt), (qraw, qt)):
                sh_ps = ps_tile("sh_ps")
                nc.tensor.matmul(sh_ps[:], lhsT=perm_sb[:], rhs=src[:, cs],
                                 start=True, stop=True, skip_group_check=True)
                t1 = work.tile([P, CH], BF16, tag="t1", name="t1")
                t2 = work.tile([P, CH], BF16, tag="t2", name="t2")
                nc.vector.tensor_mul(out=t1[:], in0=src[:, cs], in1=cos_sb[:, cs])
                nc.vector.tensor_mul(out=t2[:], in0=sh_ps[:], in1=sin_sb[:, cs])
                nc.vector.tensor_add(out=dst[:, cs], in0=t1[:], in1=t2[:])

        # --- attention work generators (pumped between V-proj s-tiles) ---
        # Generators yield (cost_emitted, gate_for_next_step); the pump holds
        # a step back until v_sb[gate] has been produced, so emitted matmuls
        # never sit blocked at the head of the in-order PE queue.
        e_tiles = {}

        def gen_scores(c):
            cs0 = c * CH
            nk = 4 * c + 4
            tiles = [[None] * nk for _ in range(2)]
            e_tiles[c] = tiles
            gate0 = 2 if c == 0 else -1
            for t in range(nk):
                m = t - 4 * c
                off = max(m, 0) * P
                w = CH - off
                for h in range(2):
                    ps_s = ps_tile("ps_s")
                    nc.tensor.matmul(
                        ps_s[:, 0:w],
                        lhsT=kt[h * HD:(h + 1) * HD, bass.ts(t, P)],
                        rhs=qt[h * HD:(h + 1) * HD, bass.ds(cs0 + off, w)],
                        start=True, stop=True, skip_group_check=True)
                    e = epool.tile([P, CH], BF16, tag="e", name=f"e{h}_{t}")
                    nc.scalar.activation(e[:, bass.ds(off, w)], ps_s[:, 0:w],
                                         mybir.ActivationFunctionType.Exp)
                    if m >= 0:
                        _mask_eng(nc).tensor_mul(out=e[:, bass.ts(m, P)],
                                                 in0=e[:, bass.ts(m, P)], in1=tri_sb[:])
                    tiles[h][t] = e
                # chunk 0 has no av() work zipped in front of it: ration its
                # pairs to ~2 per V-tile so at most ~4 score PSUM banks are
                # outstanding against the exp drain rate (8-bank pool).
                yield 2, (gate0 + (t + 1) // 2 if c == 0 else gate0)

        def gen_av(c):
            tiles = e_tiles.pop(c)
            for m in range(4):
                q_idx = 4 * c + m
                for h in range(2):
                    out_stage = outp.tile([P, 256], BF16, tag="out_stage", name="out_stage")
                    po = ps_tile("po")
                    for t in range(q_idx + 1):
                        if t > 0:
                            yield 1, t  # next matmul reads v_sb[t]
                        nc.tensor.matmul(
                            po[:, 0:257],
                            lhsT=tiles[h][t][:, bass.ts(m, P)],
                            rhs=v_sb[t][:, h * VOFF:h * VOFF + 257],
                            start=(t == 0), stop=(t == q_idx), skip_group_check=True)
                    rec = outp.tile([P, 1], F32, tag="rec", name="rec")
                    nc.vector.reciprocal(rec[:], po[:, 256:257])
                    nc.vector.tensor_scalar_mul(out_stage[:], po[:, 0:256], rec[:])
                    # per-head output DMA so the final q-block's tail is short
                    nc.sync.dma_start(
                        out=d_out[bass.ts(q_idx, P), bass.ts(h, 256)],
                        in_=out_stage[:])
                    yield 1, (0 if (h == 0 or m < 3) else -1)

        def gen_av_s(c):
            # av(c) zipped with scores(c+1): a scores pair every few av MMs.
            # A solid run of scores MMs outpaces the scalar exps that recycle
            # their PSUM banks (~730ns/exp vs ~320ns/pair) and the PE stalls
            # head-of-line; av MMs (exps long done) fill those bubbles.
            g_av = gen_av(c)
            g_s = gen_scores(c + 1) if c < NCH - 1 else None
            total_av = sum(2 * (4 * c + m + 1) for m in range(4))
            n_s = 4 * (c + 1) + 4 if g_s is not None else 0
            stride = max(2, total_av // (n_s + 1)) if g_s is not None else 10 ** 9
            i = 0
            for item in g_av:
                yield item
                i += 1
                if g_s is not None and i % stride == 0:
                    try:
                        yield next(g_s)
                    except StopIteration:
                        g_s = None
            if g_s is not None:
                for item in g_s:
                    yield item

        # queue of [gate_tile, generator]: gates are per-step (updated from
        # generator yields). scores(0) waits for 2 V tiles so its matmuls land
        # after RoPE(k0,q0) is through the vector queue.
        att_queue = [[int(os.environ.get("CHEEMS_GATE0", "2")), gen_scores(0)]]
        for c in range(NCH):
            att_queue.append([0, gen_av_s(c)])

        def pump(t_done, budget):
            emitted = 0
            while att_queue and emitted < budget:
                entry = att_queue[0]
                if entry[0] > t_done:
                    break
                try:
                    cost, gate = next(entry[1])
                    emitted += cost
                    entry[0] = gate
                except StopIteration:
                    att_queue.pop(0)
            return emitted

        # --- phase 2: V projection with attention work interleaved ---
        for t in range(NST):
            pv = ps_tile("pv")
            for k in range(NKT):
                nc.tensor.matmul(pv[:], lhsT=comb[:, k, bass.ts(t, P)], rhs=wv_sb[:, k, :],
                                 start=(k == 0), stop=(k == NKT - 1), skip_group_check=True)
            if t % 2 == 0:
                nc.scalar.copy(v_sb[t][:, 0:256], pv[:, 0:256])
                nc.scalar.copy(v_sb[t][:, VOFF:VOFF + 256], pv[:, 256:512])
            else:
                nc.vector.tensor_copy(v_sb[t][:, 0:256], pv[:, 0:256])
                nc.vector.tensor_copy(v_sb[t][:, VOFF:VOFF + 256], pv[:, 256:512])
            if t < NCH:
                # RoPE chunk t: the swap matmuls slot between V-proj tiles (so
                # pq/pk drains have landed), one chunk pair at a time to keep
                # at most 2 PSUM banks held while the DVE catches up.
                emit_rope_chunk(t)
            pump(t, 24)
        while att_queue:
            pump(NST, 10 ** 9)

    _legalize_waits(nc)
    _dedup_ldweights(nc)
    return nc


def _host_prep(hidden_states, position_ids, Wq, Wk, Wv):
    """Build the 8 per-core input maps."""
    hidden_states = np.asarray(hidden_states, dtype=np.float32)
    position_ids = np.asarray(position_ids)
    Wq = np.asarray(Wq, dtype=np.float32)
    Wk = np.asarray(Wk, dtype=np.float32)
    Wv = np.asarray(Wv, dtype=np.float32)

    scale = 1.0 / np.sqrt(HD)
    tri = np.triu(np.ones((P, P), dtype=np.float32)).astype(ml_dtypes.bfloat16)
    inv_freq = (1.0 / (THETA ** (np.arange(0, HD, 2, dtype=np.float32) / HD))).astype(np.float32)
    # rotate_half partition swap as a stationary matrix: out[d] = in[swap(d)]
    # (sign lives in the sin table); two 64-row head blocks per partition dim.
    perm = np.zeros((P, P), dtype=np.float32)
    for dd in range(P):
        perm[(dd // 64) * 64 + (dd % 64 + 32) % 64, dd] = 1.0
    perm = perm.astype(ml_dtypes.bfloat16)

    def prearrange(w):
        # [HID, N] -> [P, NKT*N]: row p holds the p-th row of every 128-row
        # k-block, so the device DMA is one contiguous 2*NKT*N-byte descriptor
        # per partition.
        n = w.shape[1]
        return np.ascontiguousarray(
            w.reshape(NKT, P, n).transpose(1, 0, 2).reshape(P, NKT * n))

    in_maps = []
    xt_b = {}
    for b in range(B):
        # comb[p, k, :] = [ X[b][:, k*P+p] (2048) | Wq[k*P+p, :]*scale (128) | Wk[k*P+p, :] (128) ]
        xt_b[b] = np.ascontiguousarray(hidden_states[b].T).reshape(NKT, P, S)
    for c in range(8):
        b, p = c // 4, c % 4
        comb = np.empty((P, NKT, CW), dtype=ml_dtypes.bfloat16)
        comb[:, :, 0:S] = xt_b[b].transpose(1, 0, 2)
        comb[:, :, S:S + P] = (Wq[:, p * P:(p + 1) * P] * scale).reshape(NKT, P, P).transpose(1, 0, 2)
        comb[:, :, S + P:S + 2 * P] = Wk[:, p * P:(p + 1) * P].reshape(NKT, P, P).transpose(1, 0, 2)
        comb = np.ascontiguousarray(comb.reshape(P, NKT * CW))
        cols = []
        for h in (2 * p, 2 * p + 1):
            for r in range(G):
                j = r * HKV + h
                cols.append(Wv[:, j * HD:(j + 1) * HD])
        wv = prearrange(np.concatenate(cols, axis=1)).astype(ml_dtypes.bfloat16)

        pos = position_ids[b].astype(np.float32)
        freqs = pos[:, None] * inv_freq[None, :]          # [S, 32]
        cos32 = np.cos(freqs).T.astype(np.float32)        # [32, S]
        sin32 = np.sin(freqs).T.astype(np.float32)
        cos64 = np.concatenate([cos32, cos32], axis=0)    # [64, S]
        sin64 = np.concatenate([-sin32, sin32], axis=0)   # sign of rotate_half folded
        cost = np.ascontiguousarray(np.concatenate([cos64, cos64], axis=0)).astype(ml_dtypes.bfloat16)  # [128, S]
        sint = np.ascontiguousarray(np.concatenate([sin64, sin64], axis=0)).astype(ml_dtypes.bfloat16)

        in_maps.append({
            "comb": comb, "wv": wv,
            "cost": cost, "sint": sint, "tri": tri, "perm": perm,
        })
    return in_maps


def kernel(hidden_states, position_ids, Wq, Wk, Wv):
    global LAST_RESULTS
    trace = bool(os.environ.get("CHEEMS_TRACE"))
    if trace:
        _install_ntff_hook()
    if "nc" not in _CACHE:
        _CACHE["nc"] = _build()
    nc = _CACHE["nc"]
    in_maps = _host_prep(hidden_states, position_ids, Wq, Wk, Wv)
    res = run_bass_kernel_spmd(nc, in_maps, core_ids=list(range(8)), trace=trace)
    LAST_RESULTS = res

    out = np.empty((B, S, HID), dtype=np.float32)
    for c in range(8):
        b, p = c // 4, c % 4
        core_out = np.asarray(res.results[c]["out"]).astype(np.float32)  # [S, 512]
        for hl, h in enumerate((2 * p, 2 * p + 1)):
            for r in range(G):
                j = r * HKV + h
                out[b, :, j * HD:(j + 1) * HD] = core_out[:, (hl * G + r) * HD:(hl * G + r + 1) * HD]
    return out.reshape(B, S, HID)



# revision 49
# speedup vs baseline: 1.1810x; 1.1810x over previous
"""Trainium2 Bass kernel for CheemsNonWoAttn (GQA attention block, no Wo).

Sharding: 8 cores = batch(2) x kv-head-pair(4). Each core handles one batch
element and 2 of the 8 kv heads (GQA: Q and K are repeated identically across
the 4 groups, so only 8 unique softmax matrices exist; V uses all 32 heads).

Per-core device program:
  Qt/Kt = Wq^T X^T, Wk^T X^T    (d on partitions; N=512 matmuls, K accum=16)
  RoPE on DVE (partition-shifted multiplies, sign folded into sin table)
  V = X Wv                      (natural [s, 512] layout)
  scores^T[k, q] = Kt^T Qt      (per head, K=64, row-tiled across array halves)
  E = exp(scores) on ACT        (no max subtraction: |scores| <~ 6, safe)
  causal mask: multiply 128x128 diagonal blocks by triangular mask (DVE)
  out = E^T V with a ones-column in V producing the softmax denominator
  out /= denom (DVE reciprocal + tensor_scalar)
"""
import os
import sys
import types
from contextlib import ExitStack

for _p in ("/opt/trn_rl_repo", "/root/.axon_site/_ro/trn_rl_repo"):
    if os.path.isdir(_p) and _p not in sys.path:
        sys.path.append(_p)

import numpy as np
import ml_dtypes

import concourse.bass as bass
import concourse.tile as tile
from concourse import mybir
from concourse.bass_utils import run_bass_kernel_spmd
from concourse.vector_clock import ScopedClock

# ---------------------------------------------------------------------------
# Patch 1: walrus rejects Drain instructions with >1 sync wait (CTRL ops have
# a single wait slot). Split the TileContext exit drain's waits across extra
# SP nops, one wait each.
def _patched_drain_and_barrier(self, tick_clock, wait_clock):
    nc = self.nc
    drain_bi = nc.sync.drain()
    wait_clock.add_sem_waits(drain_bi.ins, ScopedClock({None: tick_clock.global_clock}))
    inst = drain_bi.ins
    si = inst.sync_info
    if si is not None and si.on_wait is not None and len(si.on_wait) > 1:
        waits = list(si.on_wait)
        inst.sync_info = mybir.SyncInfo(
            on_wait=waits[:1],
            on_update=list(si.on_update) if si.on_update else [],
        )
        for w in waits[1:]:
            nbi = nc.sync.nop()
            nbi.ins.sync_info = mybir.SyncInfo(on_wait=[w], on_update=[])
    nc.all_engine_barrier()
    assert self.sems is not None
    popped = nc._tile_sem_poison_stack.pop()
    assert popped is self._sem_poison
    nc.clear_and_free_semaphores(list(self.sems.allocated().values()))
    nc.all_engine_barrier()


tile.TileContext._drain_and_barrier = _patched_drain_and_barrier


def _legalize_waits(nc):
    """This walrus build accepts at most one sync-wait per instruction.
    Split any instruction carrying N>1 waits into N-1 preceding same-engine
    nops (engines are in-order, so semantics are preserved)."""
    uid = 0
    for f in nc.m.functions:
        for blk in f.blocks:
            insts = list(blk.instructions)
            out, changed = [], False
            for inst in insts:
                si = getattr(inst, "sync_info", None)
                if si is not None and si.on_wait is not None and len(si.on_wait) > 1:
                    waits = list(si.on_wait)
                    for w in waits[:-1]:
                        uid += 1
                        out.append(mybir.InstNoOp(
                            name=f"{inst.name}_lw{uid}",
                            engine=inst.engine,
                            sync_info=mybir.SyncInfo(on_wait=[w], on_update=[]),
                            bass_nofuse=True,
                        ))
                    inst.sync_info = mybir.SyncInfo(
                        on_wait=waits[-1:],
                        on_update=list(si.on_update) if si.on_update else [],
                    )
                    changed = True
                out.append(inst)
            if changed:
                blk.instructions = out


def _dedup_ldweights(nc):
    """Clear the self-load on matmuls whose stationary operand AP is identical
    to the immediately preceding matmul in the final PE stream (walrus runs
    with ldw-opt disabled, so it reloads weights for every matmul otherwise)."""
    if os.environ.get("CHEEMS_NO_LDW_DEDUP"):
        return
    for f in nc.m.functions:
        for blk in f.blocks:
            prev_key = None
            for inst in blk.instructions:
                if not isinstance(inst, mybir.InstMatmult):
                    if isinstance(inst, mybir.InstLdweights):
                        prev_key = None
                    continue
                key = (repr(inst.ins[1]), inst.perf_mode, inst.is_transpose,
                       repr(inst.tile_position))
                if prev_key is not None and key == prev_key:
                    inst.ldweights = False
                prev_key = key


# Patch 2 (optional, for tracing): recreate the antenv.axon_hooks shim so
# run_bass_kernel_spmd(trace=True) can capture NTFF profiles under axon.
def _install_ntff_hook():
    try:
        if "antenv.axon_hooks" in sys.modules:
            return
        import antenv
        from trn_agent_boot.trn_boot import _ntff_profile_via_ctypes

        hook = _ntff_profile_via_ctypes("/opt/axon/libaxon_pjrt.so")
        mod = types.ModuleType("antenv.axon_hooks")
        mod._hook = hook
        mod.get_axon_ntff_profile_hook = lambda: mod._hook

        def _set(h):
            mod._hook = h

        mod.set_axon_ntff_profile_hook = _set
        sys.modules["antenv.axon_hooks"] = mod
        antenv.axon_hooks = mod
    except Exception:
        pass


# ---------------------------------------------------------------------------
B, S, HID = 2, 2048, 2048
NH, G = 32, 4
HD = 64          # head dim
HKV = 8          # kv heads
THETA = 10000.0
P = 128          # partitions
NKT = HID // P   # 16 k-tiles over the contraction dim
NST = S // P     # 16 s-tiles
NCH = 4          # s-chunks of 512
CH = 512
DV = 512         # v columns per core (8 heads x 64)
VROW = 528       # v tile row: [256 v | 1 one | 7 pad] x 2 heads
VOFF = 264

F32 = mybir.dt.float32
BF16 = mybir.dt.bfloat16

CW = S + 2 * P  # combined per-k-tile row: [xt 2048 | wq 128 | wk 128]

_CACHE = {}
LAST_RESULTS = None


def _mask_eng(nc):
    return nc.vector if os.environ.get("CHEEMS_MASK_VEC") else nc.gpsimd


def _build():
    nc = bass.Bass("TRN2")
    # Inputs host-prearranged to [P, ...] contiguous layouts: one 4KB+ row per
    # partition -> few large DMA descriptors. xt/wq/wk are fused into one
    # per-k-tile block ("comb": [xt 2048 | wq 128 | wk 128] per partition row)
    # so one fat DMA delivers everything the QK-proj k-iteration needs --
    # the PE can start at the first tile's arrival and never waits on a
    # separately-queued weight tensor.
    d_comb = nc.declare_dram_parameter("comb", [P, NKT * CW], BF16, isOutput=False)
    d_wv = nc.declare_dram_parameter("wv", [P, NKT * DV], BF16, isOutput=False)
    d_cos = nc.declare_dram_parameter("cost", [P, S], BF16, isOutput=False)
    d_sin = nc.declare_dram_parameter("sint", [P, S], BF16, isOutput=False)
    d_tri = nc.declare_dram_parameter("tri", [P, P], BF16, isOutput=False)
    d_perm = nc.declare_dram_parameter("perm", [P, P], BF16, isOutput=False)
    d_out = nc.declare_dram_parameter("out", [S, DV], BF16, isOutput=True)

    with tile.TileContext(nc) as tc, ExitStack() as ctx:
        pers = ctx.enter_context(tc.tile_pool(name="pers", bufs=1))
        epool = ctx.enter_context(tc.tile_pool(name="epool", bufs=56))
        work = ctx.enter_context(tc.tile_pool(name="work", bufs=1))
        outp = ctx.enter_context(tc.tile_pool(name="outp", bufs=3))
        psum = ctx.enter_context(tc.tile_pool(name="psum", bufs=8, space="PSUM"))

        def ps_tile(name):
            return psum.tile([P, CH], F32, tag="ps", bufs=8, name=name)

        # --- persistent tiles
        comb = pers.tile([P, NKT, CW], BF16, tag="comb")
        wv_sb = pers.tile([P, NKT, DV], BF16, tag="wv")
        cos_sb = pers.tile([P, S], BF16, tag="cos")
        sin_sb = pers.tile([P, S], BF16, tag="sin")
        tri_sb = pers.tile([P, P], BF16, tag="tri")

        # DMA order tuned against consumption: the QK-proj k-loop eats one
        # comb tile per ~1.7us (warm) while DMA delivers one per ~1.5us, so
        # the 16 comb tiles stream first (any gap stalls the PE *and* risks a
        # HAM idle-window re-throttle). wv/cos/sin/tri are needed only at the
        # phase-2 boundary (~38us) and ride in the tail slack, interleaved so
        # nothing arrives after its first use.
        perm_sb = pers.tile([P, P], BF16, tag="perm")
        comb_r = d_comb[:].rearrange("p (kt w) -> p kt w", w=CW)
        wv_r = d_wv[:].rearrange("p (kt n) -> p kt n", n=DV)
        nc.sync.dma_start(out=perm_sb[:], in_=d_perm[:])
        for k in range(14):
            nc.sync.dma_start(out=comb[:, k, :], in_=comb_r[:, k, :])
        nc.sync.dma_start(out=wv_sb[:, 0:4, :], in_=wv_r[:, 0:4, :])
        nc.sync.dma_start(out=comb[:, 14, :], in_=comb_r[:, 14, :])
        nc.sync.dma_start(out=wv_sb[:, 4:8, :], in_=wv_r[:, 4:8, :])
        nc.sync.dma_start(out=comb[:, 15, :], in_=comb_r[:, 15, :])
        nc.sync.dma_start(out=wv_sb[:, 8:12, :], in_=wv_r[:, 8:12, :])
        nc.sync.dma_start(out=cos_sb[:], in_=d_cos[:])
        nc.sync.dma_start(out=wv_sb[:, 12:16, :], in_=wv_r[:, 12:16, :])
        nc.sync.dma_start(out=sin_sb[:], in_=d_sin[:])
        nc.sync.dma_start(out=tri_sb[:], in_=d_tri[:])

        qt = pers.tile([P, S], BF16, tag="qt")
        kt = pers.tile([P, S], BF16, tag="kt")
        qraw = pers.tile([P, S], BF16, tag="qraw")
        kraw = pers.tile([P, S], BF16, tag="kraw")
        v_sb = [pers.tile([P, VROW], BF16, tag=f"v{t}", name=f"v{t}") for t in range(NST)]

        # ones columns for the softmax-denominator trick: written once, the
        # per-tile V copies never touch them.
        for t in range(NST):
            nc.vector.memset(v_sb[t][:, 256:257], 1.0)
            nc.vector.memset(v_sb[t][:, VOFF + 256:VOFF + 257], 1.0)

        # --- warmup: dummy matmuls wake the PE HAM clock gate (idle default is
        # 1.2GHz; ~3.4us of activity unlocks 2.4GHz) while the first DMAs land.
        # Engine preambles keep the PE/Vector queues busy until ~7.3us, and the
        # first comb tile lands ~11-13us; warmup bridges that span reading
        # perm_sb (a 32KB DMA issued first, landing ~9us, with no dependency
        # on a Vector memset stuck behind Vector's preamble).
        n_warm = int(os.environ.get("CHEEMS_WARM", "15"))
        if n_warm:
            ps_warm = ps_tile("ps_warm")
            for _ in range(n_warm):
                nc.tensor.matmul(ps_warm[:, 0:P], lhsT=perm_sb[:], rhs=perm_sb[:],
                                 start=True, stop=True, skip_group_check=True)

        # --- phase 1: Q and K projections interleaved per xt tile (8 MMs per
        # DMA arrival keeps PE duty high while the xt stream lands).
        pq = [ps_tile(f"pq{c}") for c in range(NCH)]
        pk = [ps_tile(f"pk{c}") for c in range(NCH)]
        for k in range(NKT):
            for c in range(NCH):
                nc.tensor.matmul(pq[c][:], lhsT=comb[:, k, bass.ds(S, P)],
                                 rhs=comb[:, k, bass.ts(c, CH)],
                                 start=(k == 0), stop=(k == NKT - 1), skip_group_check=True)
            for c in range(NCH):
                nc.tensor.matmul(pk[c][:], lhsT=comb[:, k, bass.ds(S + P, P)],
                                 rhs=comb[:, k, bass.ts(c, CH)],
                                 start=(k == 0), stop=(k == NKT - 1), skip_group_check=True)

        # --- drain the 8 psum banks fast (scalar: q, vector: k in parallel)
        # so V-proj matmuls restart immediately. The rotate_half partition
        # shift is a signed-swap permutation: done on the PE (lhsT = swap
        # matrix, ~0.2us per chunk, emitted interleaved with the first V-proj
        # tiles) with the sin-multiply reading the swapped chunk straight from
        # PSUM. An SBUF->SBUF DMA shift would queue behind the whole 12.5MB
        # input stream and delay RoPE (and the scores behind it) by ~7us.
        for c in range(NCH):
            cs = bass.ts(c, CH)
            nc.scalar.copy(qraw[:, cs], pq[c][:])
            nc.vector.tensor_copy(kraw[:, cs], pk[c][:])

        def emit_rope_chunk(c):
            cs = bass.ts(c, CH)
            for src, dst in ((kraw, kt), (qraw, qt)):
                sh_ps = ps_tile("sh_ps")
                nc.tensor.matmul(sh_ps[:], lhsT=perm_sb[:], rhs=src[:, cs],
                                 start=True, stop=True, skip_group_check=True)
                t1 = work.tile([P, CH], BF16, tag="t1", name="t1")
                t2 = work.tile([P, CH], BF16, tag="t2", name="t2")
                nc.vector.tensor_mul(out=t1[:], in0=src[:, cs], in1=cos_sb[:, cs])
                nc.vector.tensor_mul(out=t2[:], in0=sh_ps[:], in1=sin_sb[:, cs])
                nc.vector.tensor_add(out=dst[:, cs], in0=t1[:], in1=t2[:])

        # --- attention work generators (pumped between V-proj s-tiles) ---
        # Generators yield (cost_emitted, gate_for_next_step); the pump holds
        # a step back until v_sb[gate] has been produced, so emitted matmuls
        # never sit blocked at the head of the in-order PE queue.
        e_tiles = {}

        def gen_scores(c):
            cs0 = c * CH
            nk = 4 * c + 4
            tiles = [[None] * nk for _ in range(2)]
            e_tiles[c] = tiles
            gate0 = 2 if c == 0 else -1
            for t in range(nk):
                m = t - 4 * c
                off = max(m, 0) * P
                w = CH - off
                for h in range(2):
                    ps_s = ps_tile("ps_s")
                    nc.tensor.matmul(
                        ps_s[:, 0:w],
                        lhsT=kt[h * HD:(h + 1) * HD, bass.ts(t, P)],
                        rhs=qt[h * HD:(h + 1) * HD, bass.ds(cs0 + off, w)],
                        start=True, stop=True, skip_group_check=True)
                    e = epool.tile([P, CH], BF16, tag="e", name=f"e{h}_{t}")
                    nc.scalar.activation(e[:, bass.ds(off, w)], ps_s[:, 0:w],
                                         mybir.ActivationFunctionType.Exp)
                    if m >= 0:
                        _mask_eng(nc).tensor_mul(out=e[:, bass.ts(m, P)],
                                                 in0=e[:, bass.ts(m, P)], in1=tri_sb[:])
                    tiles[h][t] = e
                # chunk 0 has no av() work zipped in front of it: ration its
                # pairs to ~2 per V-tile so at most ~4 score PSUM banks are
                # outstanding against the exp drain rate (8-bank pool).
                yield 2, (gate0 + (t + 1) // 2 if c == 0 else gate0)

        def gen_av(c):
            tiles = e_tiles.pop(c)
            for m in range(4):
                q_idx = 4 * c + m
                for h in range(2):
                    out_stage = outp.tile([P, 256], BF16, tag="out_stage", name="out_stage")
                    po = ps_tile("po")
                    for t in range(q_idx + 1):
                        if t > 0:
                            yield 1, t  # next matmul reads v_sb[t]
                        nc.tensor.matmul(
                            po[:, 0:257],
                            lhsT=tiles[h][t][:, bass.ts(m, P)],
                            rhs=v_sb[t][:, h * VOFF:h * VOFF + 257],
                            start=(t == 0), stop=(t == q_idx), skip_group_check=True)
                    rec = outp.tile([P, 1], F32, tag="rec", name="rec")
                    nc.vector.reciprocal(rec[:], po[:, 256:257])
                    nc.vector.tensor_scalar_mul(out_stage[:], po[:, 0:256], rec[:])
                    # per-head output DMA so the final q-block's tail is short
                    nc.sync.dma_start(
                        out=d_out[bass.ts(q_idx, P), bass.ts(h, 256)],
                        in_=out_stage[:])
                    yield 1, (0 if (h == 0 or m < 3) else -1)

        def gen_av_s(c):
            # av(c) zipped with scores(c+1): a scores pair every few av MMs.
            # A solid run of scores MMs outpaces the scalar exps that recycle
            # their PSUM banks (~730ns/exp vs ~320ns/pair) and the PE stalls
            # head-of-line; av MMs (exps long done) fill those bubbles.
            g_av = gen_av(c)
            g_s = gen_scores(c + 1) if c < NCH - 1 else None
            total_av = sum(2 * (4 * c + m + 1) for m in range(4))
            n_s = 4 * (c + 1) + 4 if g_s is not None else 0
            stride = max(2, total_av // (n_s + 1)) if g_s is not None else 10 ** 9
            i = 0
            for item in g_av:
                yield item
                i += 1
                if g_s is not None and i % stride == 0:
                    try:
                        yield next(g_s)
                    except StopIteration:
                        g_s = None
            if g_s is not None:
                for item in g_s:
                    yield item

        # queue of [gate_tile, generator]: gates are per-step (updated from
        # generator yields). scores(0) waits for 2 V tiles so its matmuls land
        # after RoPE(k0,q0) is through the vector queue.
        att_queue = [[int(os.environ.get("CHEEMS_GATE0", "2")), gen_scores(0)]]
        for c in range(NCH):
            att_queue.append([0, gen_av_s(c)])

        def pump(t_done, budget):
            emitted = 0
            while att_queue and emitted < budget:
                entry = att_queue[0]
                if entry[0] > t_done:
                    break
                try:
                    cost, gate = next(entry[1])
                    emitted += cost
                    entry[0] = gate
                except StopIteration:
                    att_queue.pop(0)
            return emitted

        # --- phase 2: V projection with attention work interleaved ---
        for t in range(NST):
            pv = ps_tile("pv")
            for k in range(NKT):
                nc.tensor.matmul(pv[:], lhsT=comb[:, k, bass.ts(t, P)], rhs=wv_sb[:, k, :],
                                 start=(k == 0), stop=(k == NKT - 1), skip_group_check=True)
            if t % 2 == 0:
                nc.scalar.copy(v_sb[t][:, 0:256], pv[:, 0:256])
                nc.scalar.copy(v_sb[t][:, VOFF:VOFF + 256], pv[:, 256:512])
            else:
                nc.vector.tensor_copy(v_sb[t][:, 0:256], pv[:, 0:256])
                nc.vector.tensor_copy(v_sb[t][:, VOFF:VOFF + 256], pv[:, 256:512])
            if t < NCH:
                # RoPE chunk t: the swap matmuls slot between V-proj tiles (so
                # pq/pk drains have landed), one chunk pair at a time to keep
                # at most 2 PSUM banks held while the DVE catches up.
                emit_rope_chunk(t)
            pump(t, 24)
        while att_queue:
            pump(NST, 10 ** 9)

    _legalize_waits(nc)
    _dedup_ldweights(nc)
    return nc


def _host_prep(hidden_states, position_ids, Wq, Wk, Wv):
    """Build the 8 per-core input maps."""
    hidden_states = np.asarray(hidden_states, dtype=np.float32)
    position_ids = np.asarray(position_ids)
    Wq = np.asarray(Wq, dtype=np.float32)
    Wk = np.asarray(Wk, dtype=np.float32)
    Wv = np.asarray(Wv, dtype=np.float32)

    scale = 1.0 / np.sqrt(HD)
    tri = np.triu(np.ones((P, P), dtype=np.float32)).astype(ml_dtypes.bfloat16)
    inv_freq = (1.0 / (THETA ** (np.arange(0, HD, 2, dtype=np.float32) / HD))).astype(np.float32)
    # rotate_half partition swap as a stationary matrix: out[d] = in[swap(d)]
    # (sign lives in the sin table); two 64-row head blocks per partition dim.
    perm = np.zeros((P, P), dtype=np.float32)
    for dd in range(P):
        perm[(dd // 64) * 64 + (dd % 64 + 32) % 64, dd] = 1.0
    perm = perm.astype(ml_dtypes.bfloat16)

    def prearrange(w):
        # [HID, N] -> [P, NKT*N]: row p holds the p-th row of every 128-row
        # k-block, so the device DMA is one contiguous 2*NKT*N-byte descriptor
        # per partition.
        n = w.shape[1]
        return np.ascontiguousarray(
            w.reshape(NKT, P, n).transpose(1, 0, 2).reshape(P, NKT * n))

    in_maps = []
    xt_b = {}
    for b in range(B):
        # comb[p, k, :] = [ X[b][:, k*P+p] (2048) | Wq[k*P+p, :]*scale (128) | Wk[k*P+p, :] (128) ]
        xt_b[b] = np.ascontiguousarray(hidden_states[b].T).reshape(NKT, P, S)
    for c in range(8):
        b, p = c // 4, c % 4
        comb = np.empty((P, NKT, CW), dtype=ml_dtypes.bfloat16)
        comb[:, :, 0:S] = xt_b[b].transpose(1, 0, 2)
        comb[:, :, S:S + P] = (Wq[:, p * P:(p + 1) * P] * scale).reshape(NKT, P, P).transpose(1, 0, 2)
        comb[:, :, S + P:S + 2 * P] = Wk[:, p * P:(p + 1) * P].reshape(NKT, P, P).transpose(1, 0, 2)
        comb = np.ascontiguousarray(comb.reshape(P, NKT * CW))
        cols = []
        for h in (2 * p, 2 * p + 1):
            for r in range(G):
                j = r * HKV + h
                cols.append(Wv[:, j * HD:(j + 1) * HD])
        wv = prearrange(np.concatenate(cols, axis=1)).astype(ml_dtypes.bfloat16)

        pos = position_ids[b].astype(np.float32)
        freqs = pos[:, None] * inv_freq[None, :]          # [S, 32]
        cos32 = np.cos(freqs).T.astype(np.float32)        # [32, S]
        sin32 = np.sin(freqs).T.astype(np.float32)
        cos64 = np.concatenate([cos32, cos32], axis=0)    # [64, S]
        sin64 = np.concatenate([-sin32, sin32], axis=0)   # sign of rotate_half folded
        cost = np.ascontiguousarray(np.concatenate([cos64, cos64], axis=0)).astype(ml_dtypes.bfloat16)  # [128, S]
        sint = np.ascontiguousarray(np.concatenate([sin64, sin64], axis=0)).astype(ml_dtypes.bfloat16)

        in_maps.append({
            "comb": comb, "wv": wv,
            "cost": cost, "sint": sint, "tri": tri, "perm": perm,
        })
    return in_maps


def kernel(hidden_states, position_ids, Wq, Wk, Wv):
    global LAST_RESULTS
    trace = bool(os.environ.get("CHEEMS_TRACE"))
    if trace:
        _install_ntff_hook()
    if "nc" not in _CACHE:
        _CACHE["nc"] = _build()
    nc = _CACHE["nc"]
    in_maps = _host_prep(hidden_states, position_ids, Wq, Wk, Wv)
    res = run_bass_kernel_spmd(nc, in_maps, core_ids=list(range(8)), trace=trace)
    LAST_RESULTS = res

    out = np.empty((B, S, HID), dtype=np.float32)
    for c in range(8):
        b, p = c // 4, c % 4
        core_out = np.asarray(res.results[c]["out"]).astype(np.float32)  # [S, 512]
        for hl, h in enumerate((2 * p, 2 * p + 1)):
            for r in range(G):
                j = r * HKV + h
                out[b, :, j * HD:(j + 1) * HD] = core_out[:, (hl * G + r) * HD:(hl * G + r + 1) * HD]
    return out.reshape(B, S, HID)

